# revision 12
# baseline (speedup 1.0000x reference)
"""KNN (B=4, N=M=8192, C=3, k=16) Bass kernel for 8 trn2 NeuronCores.

Sharding: core c handles batch b=c//2, query rows [ (c%2)*4096, +4096 ).
Each core computes, for its 4096 queries, squared distances to all 8192
points of its batch via a K=4 TensorE matmul producing
    psum[n, m] = 2*x1[n]
x2[m] - |x2[m]|^2   ( = |x1[n]|^2 - dist2[n,m] )
which, per query row, orders identically to -dist2 (|x1|^2 is a
per-row constant).

Top-16 per row (DVE), blocked to minimize vector-engine cycles:
  stage 1: max8 over 8 blocks of 1024  -> cand[128, 64] block top-8s
  stage 2: max8 / match_replace / max8 on cand -> v16 (top-16 values)
  stage 3: two max_index scans of the full stream (8 needles each)
           recover the 16 global indices
Values = sqrt(|x1|^2 - v) on the 16 winners only (ScalarE), which
reproduces the reference's cancellation behaviour including NaNs for
near-coincident points.

Exactness: stage 2 candidates miss a winner only if one 1024-block
holds >=8 of the true top-16; the kernel flags rows where
max_b(block 8th-best) >= 16th candidate value.  Exact f32 value ties
(where max_index's first-occurrence semantics can emit duplicate
indices, and where top_k's post-sqrt tie order can differ from our
pre-sqrt order) are caught host-side as tied/duplicate/out-of-range
outputs.  All flagged rows are recomputed on the host with the exact
reference formula (~600 rows per run, vectorized numpy).
"""

import numpy as np

import concourse.bass as bass  # noqa: F401  (engine classes register)
import concourse.bacc as bacc
from concourse import mybir, tile
from concourse.bass_utils import run_bass_kernel_spmd

B, N, M, C, K = 4, 8192, 8192, 3, 16
NCORES = 8
NLOC = B * N // NCORES      # 4096 query rows per core
P = 128                     # partition dim (queries per tile)
MB = 512                    # matmul moving-free chunk (one PSUM bank)
NMB = M // MB               # 16 chunks
SB = 1024                   # stage-1 max8 block size
NSB = M // SB               # 8 blocks
NEG_FILL = -3.0e38

_cached_nc = {}


def build(nt=NLOC // P):
    """Build + compile the SPMD program (nt row-tiles of 128 queries)."""
    if nt in _cached_nc:
        return _cached_nc[nt]
    f32 = mybir.dt.float32
    u32 = mybir.dt.uint32
    AX = mybir.AxisListType
    ALU = mybir.AluOpType
    nc = bacc.Bacc("TRN2", target_bir_lowering=False, debug=False,
                   num_devices=NCORES)
    # single packed input / output tensors: every extra PJRT operand costs
    # ~8 extra axon shard round-trips (~100 ms) per call, dwarfing exec time
    A1, R2 = 4 * NLOC, 4 * M
    flat_d = nc.dram_tensor("flat", [A1 + R2 + NLOC], f32,
                            kind="ExternalInput")
    out_d = nc.dram_tensor("out", [nt, P, 2 * K + 1], u32,
                           kind="ExternalOutput")
    a1_d = flat_d[0:A1].rearrange("(a b) -> a b", b=NLOC)
    r2_d = flat_d[A1:A1 + R2].rearrange("(a b) -> a b", b=M)
    n1_d = flat_d[A1 + R2:A1 + R2 + NLOC].rearrange("(a b) -> a b",
                                                    b=NLOC // P)

    with tile.TileContext(nc) as tc:
        with (
            tc.tile_pool(name="const", bufs=1) as constp,
            tc.tile_pool(name="psum", bufs=8, space="PSUM") as psump,
            tc.tile_pool(name="work", bufs=2) as workp,
            tc.tile_pool(name="cand", bufs=1) as candp,
            tc.tile_pool(name="outp", bufs=3) as outp,
        ):
            r2_sb = constp.tile([4, M], f32)
            nc.sync.dma_start(out=r2_sb[:], in_=r2_d)
            a1_sb = constp.tile([4, NLOC], f32)
            nc.sync.dma_start(out=a1_sb[:], in_=a1_d)
            n1_sb = constp.tile([P, NLOC // P], f32)
            nc.sync.dma_start(out=n1_sb[:], in_=n1_d)

            for t in range(nt):
                # ---- distances:  neg[p, m] = 2*x1.x2 - |x2|^2 ----
                neg = workp.tile([P, M], f32, tag="neg")
                for j in range(NMB):
                    ps = psump.tile([P, MB], f32, tag="ps")
                    nc.tensor.matmul(
                        ps[:],
                        a1_sb[:, t * P:(t + 1) * P],
                        r2_sb[:, j * MB:(j + 1) * MB],
                        start=True, stop=True,
                    )
                    nc.scalar.copy(out=neg[:, j * MB:(j + 1) * MB], in_=ps[:])

                # ---- stage 1: per-block top-8 ----
                cand = candp.tile([P, NSB * 8], f32, tag="cand")
                for b in range(NSB):
                    nc.vector.max(cand[:, b * 8:(b + 1) * 8],
                                  neg[:, b * SB:(b + 1) * SB])

                # ---- stage 2: merge candidates -> top-16 values ----
                crep = candp.tile([P, NSB * 8], f32, tag="crep")
                v16 = outp.tile([P, K], f32, tag="v16")
                nc.vector.max(v16[:, 0:8], cand[:])
                nc.vector.match_replace(crep[:], v16[:, 0:8], cand[:],
                                        NEG_FILL)
                nc.vector.max(v16[:, 8:16], crep[:])

                # completeness flag: any block's 8th-best still >= 16th cand
                wmax = outp.tile([P, 1], f32, tag="wmax")
                cand_v = cand[:].rearrange("p (b e) -> p e b", e=8)
                nc.vector.tensor_reduce(wmax[:], cand_v[:, 7:8, :], AX.XY,
                                        ALU.max)
                flg = outp.tile([P, 1], f32, tag="flg")
                nc.vector.tensor_tensor(out=flg[:], in0=wmax[:],
                                        in1=v16[:, 15:16], op=ALU.is_ge)
                nc.sync.dma_start(out=out_d[t, :, 2 * K:2 * K + 1],
                                  in_=flg[:].bitcast(u32))

                # ---- stage 3: global indices (full-stream scan per group;
                # cross-group duplicate needles are host-flagged) ----
                i16u = outp.tile([P, K], u32, tag="i16u")
                for g in range(2):
                    nc.vector.max_index(
                        i16u[:, g * 8:(g + 1) * 8],
                        v16[:, g * 8:(g + 1) * 8],
                        neg[:],
                    )
                nc.sync.dma_start(out=out_d[t, :, K:2 * K], in_=i16u[:])

                # ---- values: dist = sqrt(|x1|^2 - v) ----
                d16 = outp.tile([P, K], f32, tag="d16")
                nc.scalar.activation(
                    d16[:], v16[:], mybir.ActivationFunctionType.Sqrt,
                    bias=n1_sb[:, t:t + 1], scale=-1.0,
                )
                nc.sync.dma_start(out=out_d[t, :, 0:K],
                                  in_=d16[:].bitcast(u32))

    nc.compile()
    _cached_nc[nt] = nc
    return nc


def make_in_maps(xyz1, xyz2):
    in_maps = []
    for c in range(NCORES):
        b, h = c // 2, c % 2
        x1 = xyz1[b, h * NLOC:(h + 1) * NLOC]        # [NLOC, 3]
        x2 = xyz2[b]                                  # [M, 3]
        a1t = np.empty((4, NLOC), np.float32)
        a1t[0:3] = 2.0 * x1.T
        a1t[3] = -1.0
        n1 = (x1 * x1).sum(-1)                        # [NLOC]
        r2 = np.empty((4, M), np.float32)
        r2[0:3] = x2.T
        r2[3] = (x2 * x2).sum(-1)
        n1h = np.ascontiguousarray(n1.reshape(-1, P).T)
        in_maps.append({
            "flat": np.concatenate(
                [a1t.ravel(), r2.ravel(), n1h.ravel()]).astype(np.float32),
        })
    return in_maps


def _fixup(vals, idx, flags, xyz1, xyz2):
    """Host fallback: recompute rows the device flagged as suspect with
    the exact reference formula (stable top-k, NaN-first like lax.top_k)."""
    suspect = flags > 0.5
    suspect |= (idx >= M).any(-1) | (idx < 0).any(-1)
    sidx = np.sort(idx, axis=-1)
    suspect |= (sidx[..., 1:] == sidx[..., :-1]).any(-1)
    suspect |= (vals[..., 1:] == vals[..., :-1]).any(-1)
    nrows = 0
    for b in range(vals.shape[0]):
        ns = np.flatnonzero(suspect[b])
        if ns.size == 0:
            continue
        nrows += ns.size
        x1 = xyz1[b, ns]                                     # [R, 3]
        x2 = xyz2[b]                                         # [M, 3]
        d2 = (-2.0 * (x1 @ x2.T) + (x1 * x1).sum(-1)[:, None]
              + (x2 * x2).sum(-1)[None, :]).astype(np.float32)
        dist = np.sqrt(d2)
        key = np.where(np.isnan(dist), np.float32(-np.inf), dist)
        order = np.argsort(key, axis=1, kind="stable")[:, :K]
        vals[b, ns] = np.take_along_axis(dist, order, axis=1)
        idx[b, ns] = order.astype(np.int32)
    return nrows


def run(xyz1, xyz2, **spmd_kwargs):
    nc = build()
    in_maps = make_in_maps(xyz1, xyz2)
    res = run_bass_kernel_spmd(nc, in_maps, list(range(NCORES)), **spmd_kwargs)
    vals = np.empty((B, N, K), np.float32)
    idx = np.empty((B, N, K), np.int32)
    flags = np.empty((B, N), np.float32)
    for c in range(NCORES):
        b, h = c // 2, c % 2
        sl = slice(h * NLOC, (h + 1) * NLOC)
        buf = res.results[c]["out"].reshape(NLOC, 2 * K + 1)
        vals[b, sl] = np.ascontiguousarray(buf[:, 0:K]).view(np.float32)
        idx[b, sl] = np.minimum(buf[:, K:2 * K],
                                np.uint32(2 ** 31 - 1)).astype(np.int32)
        flags[b, sl] = (buf[:, 2 * K] != 0).astype(np.float32)
    nfix = _fixup(vals, idx, flags, xyz1, xyz2)
    return (vals, idx), res, nfix


def kernel(xyz1, xyz2, k):
    xyz1 = np.asarray(xyz1, dtype=np.float32)
    xyz2 = np.asarray(xyz2, dtype=np.float32)
    assert int(k) == K, f"kernel hardcodes k={K}, got {k}"
    assert xyz1.shape == (B, N, C) and xyz2.shape == (B, M, C)
    (vals, idx), _, _ = run(xyz1, xyz2)
    return vals, idx


# revision 14
# speedup vs baseline: 1.0062x; 1.0062x over previous
"""KNN (B=4, N=M=8192, C=3, k=16) Bass kernel for 8 trn2 NeuronCores.

Sharding: core c handles batch b=c//2, query rows [ (c%2)*4096, +4096 ).
Each core computes, for its 4096 queries, squared distances to all 8192
points of its batch via a K=4 TensorE matmul producing
    psum[n, m] = 2*x1[n]
x2[m] - |x2[m]|^2   ( = |x1[n]|^2 - dist2[n,m] )
which, per query row, orders identically to -dist2 (|x1|^2 is a
per-row constant).

Top-16 per row (DVE), blocked to minimize vector-engine cycles:
  stage 1: max8 over 8 blocks of 1024  -> cand[128, 64] block top-8s
  stage 2: max8 / match_replace / max8 on cand -> v16 (top-16 values)
  stage 3: two max_index scans of the full stream (8 needles each)
           recover the 16 global indices
Values = sqrt(|x1|^2 - v) on the 16 winners only (ScalarE), which
reproduces the reference's cancellation behaviour including NaNs for
near-coincident points.

Exactness: stage 2 candidates miss a winner only if one 1024-block
holds >=8 of the true top-16; the kernel flags rows where
max_b(block 8th-best) >= 16th candidate value.  Exact f32 value ties
(where max_index's first-occurrence semantics can emit duplicate
indices, and where top_k's post-sqrt tie order can differ from our
pre-sqrt order) are caught host-side as tied/duplicate/out-of-range
outputs.  All flagged rows are recomputed on the host with the exact
reference formula (~600 rows per run, vectorized numpy).
"""

import numpy as np

import concourse.bass as bass  # noqa: F401  (engine classes register)
import concourse.bacc as bacc
from concourse import mybir, tile
from concourse.bass_utils import run_bass_kernel_spmd

B, N, M, C, K = 4, 8192, 8192, 3, 16
NCORES = 8
NLOC = B * N // NCORES      # 4096 query rows per core
P = 128                     # partition dim (queries per tile)
MB = 512                    # matmul moving-free chunk (one PSUM bank)
NMB = M // MB               # 16 chunks
SB = 1024                   # stage-1 max8 block size
NSB = M // SB               # 8 blocks
NEG_FILL = -3.0e38

_cached_nc = {}


def build(nt=NLOC // P):
    """Build + compile the SPMD program (nt row-tiles of 128 queries)."""
    if nt in _cached_nc:
        return _cached_nc[nt]
    f32 = mybir.dt.float32
    u32 = mybir.dt.uint32
    u16 = mybir.dt.uint16
    AX = mybir.AxisListType
    ALU = mybir.AluOpType
    nc = bacc.Bacc("TRN2", target_bir_lowering=False, debug=False,
                   num_devices=NCORES)
    # single packed input / output tensors: every extra PJRT operand costs
    # ~8 extra axon shard round-trips (~100 ms) per call, dwarfing exec time
    A1, R2 = 4 * NLOC, 4 * M
    flat_d = nc.dram_tensor("flat", [A1 + R2 + NLOC], f32,
                            kind="ExternalInput")
    OC = K + K // 2 + 1      # 16 f32 vals + 16 u16 idx + 1 flag, as u32
    out_d = nc.dram_tensor("out", [nt, P, OC], u32, kind="ExternalOutput")
    a1_d = flat_d[0:A1].rearrange("(a b) -> a b", b=NLOC)
    r2_d = flat_d[A1:A1 + R2].rearrange("(a b) -> a b", b=M)
    n1_d = flat_d[A1 + R2:A1 + R2 + NLOC].rearrange("(a b) -> a b",
                                                    b=NLOC // P)

    with tile.TileContext(nc) as tc:
        with (
            tc.tile_pool(name="const", bufs=1) as constp,
            tc.tile_pool(name="psum", bufs=2, space="PSUM") as psump,
            tc.tile_pool(name="work", bufs=2) as workp,
            tc.tile_pool(name="cand", bufs=1) as candp,
            tc.tile_pool(name="outp", bufs=3) as outp,
        ):
            r2_sb = constp.tile([4, M], f32)
            nc.sync.dma_start(out=r2_sb[:], in_=r2_d)
            a1_sb = constp.tile([4, NLOC], f32)
            nc.sync.dma_start(out=a1_sb[:], in_=a1_d)
            n1_sb = constp.tile([P, NLOC // P], f32)
            nc.sync.dma_start(out=n1_sb[:], in_=n1_d)

            for t in range(nt):
                # ---- distances:  neg[p, m] = 2*x1.x2 - |x2|^2 ----
                neg = workp.tile([P, M], f32, tag="neg")
                for j0 in range(NMB // 4):
                    ps = psump.tile([P, 4 * MB], f32, tag="ps")
                    for j1 in range(4):
                        j = j0 * 4 + j1
                        nc.tensor.matmul(
                            ps[:, j1 * MB:(j1 + 1) * MB],
                            a1_sb[:, t * P:(t + 1) * P],
                            r2_sb[:, j * MB:(j + 1) * MB],
                            start=True, stop=True,
                        )
                    nc.scalar.copy(out=neg[:, j0 * 4 * MB:(j0 + 1) * 4 * MB],
                                   in_=ps[:])

                # ---- stage 1: per-block top-8 ----
                cand = candp.tile([P, NSB * 8], f32, tag="cand")
                for b in range(NSB):
                    nc.vector.max(cand[:, b * 8:(b + 1) * 8],
                                  neg[:, b * SB:(b + 1) * SB])

                # ---- stage 2: merge candidates -> top-16 values ----
                crep = candp.tile([P, NSB * 8], f32, tag="crep")
                v16 = outp.tile([P, K], f32, tag="v16")
                nc.vector.max(v16[:, 0:8], cand[:])
                nc.vector.match_replace(crep[:], v16[:, 0:8], cand[:],
                                        NEG_FILL)
                nc.vector.max(v16[:, 8:16], crep[:])

                # completeness flag: any block's 8th-best still >= 16th cand
                wmax = outp.tile([P, 1], f32, tag="wmax")
                cand_v = cand[:].rearrange("p (b e) -> p e b", e=8)
                nc.vector.tensor_reduce(wmax[:], cand_v[:, 7:8, :], AX.XY,
                                        ALU.max)
                flg = outp.tile([P, 1], f32, tag="flg")
                nc.vector.tensor_tensor(out=flg[:], in0=wmax[:],
                                        in1=v16[:, 15:16], op=ALU.is_ge)
                nc.sync.dma_start(out=out_d[t, :, OC - 1:OC],
                                  in_=flg[:].bitcast(u32))

                # ---- stage 3: global indices (full-stream scan per group;
                # cross-group duplicate needles are host-flagged) ----
                i16u = outp.tile([P, K], u16, tag="i16u")
                for g in range(2):
                    nc.vector.max_index(
                        i16u[:, g * 8:(g + 1) * 8],
                        v16[:, g * 8:(g + 1) * 8],
                        neg[:],
                    )
                nc.sync.dma_start(out=out_d[t, :, K:K + K // 2],
                                  in_=i16u[:].bitcast(u32))

                # ---- values: dist = sqrt(|x1|^2 - v) ----
                d16 = outp.tile([P, K], f32, tag="d16")
                nc.scalar.activation(
                    d16[:], v16[:], mybir.ActivationFunctionType.Sqrt,
                    bias=n1_sb[:, t:t + 1], scale=-1.0,
                )
                nc.sync.dma_start(out=out_d[t, :, 0:K],
                                  in_=d16[:].bitcast(u32))

    nc.compile()
    _cached_nc[nt] = nc
    return nc


def make_in_maps(xyz1, xyz2):
    in_maps = []
    for c in range(NCORES):
        b, h = c // 2, c % 2
        x1 = xyz1[b, h * NLOC:(h + 1) * NLOC]        # [NLOC, 3]
        x2 = xyz2[b]                                  # [M, 3]
        a1t = np.empty((4, NLOC), np.float32)
        a1t[0:3] = 2.0 * x1.T
        a1t[3] = -1.0
        n1 = (x1 * x1).sum(-1)                        # [NLOC]
        r2 = np.empty((4, M), np.float32)
        r2[0:3] = x2.T
        r2[3] = (x2 * x2).sum(-1)
        n1h = np.ascontiguousarray(n1.reshape(-1, P).T)
        in_maps.append({
            "flat": np.concatenate(
                [a1t.ravel(), r2.ravel(), n1h.ravel()]).astype(np.float32),
        })
    return in_maps


def _fixup(vals, idx, flags, xyz1, xyz2):
    """Host fallback: recompute rows the device flagged as suspect with
    the exact reference formula (stable top-k, NaN-first like lax.top_k)."""
    suspect = flags > 0.5
    suspect |= (idx >= M).any(-1) | (idx < 0).any(-1)
    sidx = np.sort(idx, axis=-1)
    suspect |= (sidx[..., 1:] == sidx[..., :-1]).any(-1)
    suspect |= (vals[..., 1:] == vals[..., :-1]).any(-1)
    nrows = 0
    for b in range(vals.shape[0]):
        ns = np.flatnonzero(suspect[b])
        if ns.size == 0:
            continue
        nrows += ns.size
        x1 = xyz1[b, ns]                                     # [R, 3]
        x2 = xyz2[b]                                         # [M, 3]
        d2 = (-2.0 * (x1 @ x2.T) + (x1 * x1).sum(-1)[:, None]
              + (x2 * x2).sum(-1)[None, :]).astype(np.float32)
        dist = np.sqrt(d2)
        key = np.where(np.isnan(dist), np.float32(-np.inf), dist)
        order = np.argsort(key, axis=1, kind="stable")[:, :K]
        vals[b, ns] = np.take_along_axis(dist, order, axis=1)
        idx[b, ns] = order.astype(np.int32)
    return nrows


def run(xyz1, xyz2, **spmd_kwargs):
    nc = build()
    in_maps = make_in_maps(xyz1, xyz2)
    res = run_bass_kernel_spmd(nc, in_maps, list(range(NCORES)), **spmd_kwargs)
    vals = np.empty((B, N, K), np.float32)
    idx = np.empty((B, N, K), np.int32)
    flags = np.empty((B, N), np.float32)
    for c in range(NCORES):
        b, h = c // 2, c % 2
        sl = slice(h * NLOC, (h + 1) * NLOC)
        buf = res.results[c]["out"].reshape(NLOC, K + K // 2 + 1)
        vals[b, sl] = np.ascontiguousarray(buf[:, 0:K]).view(np.float32)
        idx[b, sl] = np.ascontiguousarray(
            buf[:, K:K + K // 2]).view(np.uint16).astype(np.int32)
        flags[b, sl] = (buf[:, K + K // 2] != 0).astype(np.float32)
    nfix = _fixup(vals, idx, flags, xyz1, xyz2)
    return (vals, idx), res, nfix


def kernel(xyz1, xyz2, k):
    xyz1 = np.asarray(xyz1, dtype=np.float32)
    xyz2 = np.asarray(xyz2, dtype=np.float32)
    assert int(k) == K, f"kernel hardcodes k={K}, got {k}"
    assert xyz1.shape == (B, N, C) and xyz2.shape == (B, M, C)
    (vals, idx), _, _ = run(xyz1, xyz2)
    return vals, idx
